# revision 34
# baseline (speedup 1.0000x reference)
"""Trainium2 Bass kernel for nn_MHA_63118839382398.

Full MHA block: fused QKV projection, per-head RMSNorm on q/k, rotate-half
RoPE, causal softmax attention, output projection.

Sharding over 8 NeuronCores: core c handles batch b = c//2 and heads
[8*(c%2), 8*(c%2)+8) (tensor parallel over head halves within a batch
pair). Each core computes a partial out-projection; a 2-rank
ReduceScatter (bf16) over each pair sums the partials and leaves each
core with half of that batch's token rows, which the host reassembles.

Layout strategy (all transposed, feats x tokens), so every matmul
contraction sits on the partition axis with no on-chip transposes except
V (cheap PE-mode 128x128 transposes):
  - xT (dmodel, ntok) per batch, host-pretransposed, bf16.
  - qT/kT = W @ xT  -> (head_dim, ntok) psum f32.
  - RMS factors: sumsq via indicator matmul, rsqrt as Exp(-0.5*Ln(.))
    (both functions live in one pinned ACT table set), broadcast over
    the 128 rows via a tiny indicator matmul.
  - RoPE rotate-half swap via a PE permutation matmul; the qn_w/kn_w
    gains are folded into the host-built cos/sin tables.
  - scores computed transposed: S^T = K @ Q^T (keys part, queries free),
    bf16, causal blocks only; exp on ACT with the 1/sqrt(d) scale folded
    in; softmax denominator via a ones-column appended to the V
    stationary operand of the P@V matmul; 1/sum as Exp(-Ln(den)),
    broadcast via a ones-matmul, applied to the accumulated P@V.

Scheduling: wave w+1's projections are emitted interleaved with wave
w's attention chunks (keeps the PE stream dense for the HAM clock);
out-projection + a per-query-chunk 2-rank ReduceScatter overlap the
last wave; initial DMAs are split across the sync/act/gpsimd queues.
"""

import sys

if "/opt/trn_rl_repo" not in sys.path:
    sys.path.insert(0, "/opt/trn_rl_repo")

import numpy as np
import ml_dtypes

import concourse.bass as bass
import concourse.tile as tile
from concourse import bacc, mybir
from concourse.bass_utils import run_bass_kernel_spmd
from concourse.masks import make_identity

# Problem constants (hardcoded per harness contract).
B = 4
N = 2048
D_MODEL = 1024
N_HEADS = 16
D_HEAD = 64
ROPE_BASE = 10000.0
EPS = float(np.finfo(np.float32).eps)
N_CORES = 8

HPC = N_HEADS // 2          # heads per core = 8
WAVES = HPC // 2            # head-pair waves = 4
TOKCH = 512                 # token chunk for projections / q chunks
NT = N // TOKCH             # 4
QT = 128                    # query tile for mask classification
NQT = N // QT               # 16
KB = 128                    # key block
NKB = N // KB               # 16
DC = 128                    # dmodel chunk
NDC = D_MODEL // DC         # 8

F32 = mybir.dt.float32
BF16 = mybir.dt.bfloat16
BF = ml_dtypes.bfloat16

ACT = mybir.ActivationFunctionType

_CACHE = {}


def _pin_act_tables(arch):
    """Steer bacc's ACT-table-set choice to natural_log_exp_and_others.

    The insertion pass picks the first set containing each activation's
    function; Exp and Ln resolve to different sets by default, causing a
    ~2.7us table reload per alternation. Removing our four functions
    from every other set's *selection metadata* (runtime tables in
    act_info.json are untouched, and set ids keep their positions) makes
    all of Copy/Square/Ln/Exp resolve to the one set that has them all.
    """
    from concourse.hw_specs import get_activation_tables

    tables = get_activation_tables(arch)  # cached by reference
    keep = "natural_log_exp_and_others"
    if keep not in tables:
        return
    ours = {ACT.Copy, ACT.Square, ACT.Ln, ACT.Exp, ACT.Identity}
    for name, fns in tables.items():
        if name != keep:
            fns -= ours


def _classify_mask(mask):
    """Per (key-block, query-tile) classification of the mask.

    Returns (state[NKB][NQT], patterns): state is 'skip' (all masked),
    'full' (none masked), or an index into patterns: unique (128,128)
    bf16 0/1 tiles indexed [key, query]."""
    mask = np.asarray(mask)
    assert mask.shape == (N, N)
    patterns = []
    pat_keys = {}
    state = [[None] * NQT for _ in range(NKB)]
    for kb in range(NKB):
        for qt in range(NQT):
            blk = mask[qt * QT : (qt + 1) * QT, kb * KB : (kb + 1) * KB]
            if blk.all():
                state[kb][qt] = "skip"
            elif not blk.any():
                state[kb][qt] = "full"
            else:
                tileq = (~blk.T).astype(BF)
                key = tileq.tobytes()
                if key not in pat_keys:
                    pat_keys[key] = len(patterns)
                    patterns.append(tileq)
                state[kb][qt] = pat_keys[key]
    return state, patterns


def _build_program(state, n_patterns):
    """Build the SPMD Bass program (same graph on all 8 cores)."""
    nc = bacc.Bacc(
        "TRN2", target_bir_lowering=False, debug=False, num_devices=N_CORES
    )
    _pin_act_tables(nc.m.arch)

    p_xt = nc.dram_tensor("xt", [D_MODEL, N], BF16, kind="ExternalInput").ap()
    p_wqk = nc.dram_tensor("wqk", [128, 2, WAVES, NDC, 128], BF16, kind="ExternalInput").ap()
    p_wv = nc.dram_tensor("wv", [128, WAVES, NDC, 128], BF16, kind="ExternalInput").ap()
    p_wo = nc.dram_tensor("wo", [128, 4, D_MODEL], BF16, kind="ExternalInput").ap()
    p_rope = nc.dram_tensor("rope", [128, 4, N], BF16, kind="ExternalInput").ap()
    p_wfold = nc.dram_tensor("wfold", [2, 128], BF16, kind="ExternalInput").ap()
    p_ind2 = nc.dram_tensor("ind2", [128, 2], BF16, kind="ExternalInput").ap()
    p_pswap = nc.dram_tensor("pswap", [128, 128], BF16, kind="ExternalInput").ap()
    if n_patterns:
        p_pat = nc.dram_tensor(
            "pat", [128, n_patterns, 128], BF16, kind="ExternalInput"
        ).ap()
    p_out = nc.dram_tensor("out", [N // 2, D_MODEL], F32, kind="ExternalOutput").ap()

    y_parts = [
        nc.dram_tensor(f"y_part{qc}", [TOKCH, D_MODEL], BF16) for qc in range(NT)
    ]
    rs_outs = [
        nc.dram_tensor(f"rs_out{qc}", [256, D_MODEL], BF16) for qc in range(NT)
    ]

    QPC = TOKCH // QT  # query tiles per chunk = 4
    n_kb = [0] * NT
    qlo_t = {}
    for qc in range(NT):
        for kb in range(NKB):
            sub = [state[kb][qc * QPC + j] for j in range(QPC)]
            if all(s == "skip" for s in sub):
                continue
            n_kb[qc] = max(n_kb[qc], kb + 1)
            lead = 0
            while sub[lead] == "skip":
                lead += 1
            qlo_t[(qc, kb)] = lead

    with tile.TileContext(nc) as tc:
        import contextlib

        ctx = contextlib.ExitStack()
        with ctx:
            singles = ctx.enter_context(tc.tile_pool(name="singles", bufs=1))
            wpool = ctx.enter_context(tc.tile_pool(name="wpool", bufs=1))
            wavep = ctx.enter_context(tc.tile_pool(name="wavep", bufs=2))
            facp = ctx.enter_context(tc.tile_pool(name="facp", bufs=2))
            work = ctx.enter_context(tc.tile_pool(name="work", bufs=2))
            espool = ctx.enter_context(tc.tile_pool(name="es", bufs=4))
            epi = ctx.enter_context(tc.tile_pool(name="epi", bufs=2))
            outp = ctx.enter_context(tc.tile_pool(name="outp", bufs=2))

            pp = ctx.enter_context(tc.tile_pool(name="pp", bufs=2, space="PSUM"))
            ps = ctx.enter_context(tc.tile_pool(name="ps", bufs=3, space="PSUM"))
            pv = ctx.enter_context(tc.tile_pool(name="pv", bufs=3, space="PSUM"))

            # ---- resident constants -------------------------------------
            xt_sb = singles.tile([128, NDC, N], BF16)
            for t in range(NT):
                for dc in range(NDC):
                    nc.sync.dma_start(
                        out=xt_sb[:, dc, t * TOKCH : (t + 1) * TOKCH],
                        in_=p_xt[
                            dc * DC : (dc + 1) * DC, t * TOKCH : (t + 1) * TOKCH
                        ],
                    )
            wqk_sb = singles.tile([128, 2, WAVES, NDC, 128], BF16)
            wv_all = singles.tile([128, WAVES, NDC, 128], BF16)
            rope_sb = singles.tile([128, 4, N], BF16)  # cq, sq, ck, sk
            ident = singles.tile([128, 128], BF16)
            make_identity(nc, ident)
            ones_col = singles.tile([128, 64], BF16)
            nc.vector.memset(ones_col, 1.0)
            eps_sb = singles.tile([128, 1], F32)
            nc.vector.memset(eps_sb, EPS)
            wfold = singles.tile([2, 128], BF16)
            nc.sync.dma_start(out=wfold, in_=p_wfold)
            pswap = singles.tile([128, 128], BF16)
            nc.sync.dma_start(out=pswap, in_=p_pswap)
            ind2 = singles.tile([128, 2], BF16)
            nc.sync.dma_start(out=ind2, in_=p_ind2)
            if n_patterns:
                pat_sb = singles.tile([128, n_patterns, 128], BF16)
            yt_sb = singles.tile([128, WAVES, N], BF16)
            wo_sb = singles.tile([128, 4, D_MODEL], BF16)

            def emit_A_head(w):
                t_ = {}
                raw_w = wavep.tile([128, 2, N], BF16, tag="raw", name="raw_w")
                q_rot = wavep.tile([128, N], BF16, tag="qrot", name="q_rot")
                k_rot = wavep.tile([128, N], BF16, tag="krot", name="k_rot")
                v_sb = wavep.tile([128, NKB, 130], BF16, tag="v", name="v_sb")
                nc.vector.memset(v_sb[:, :, 64:65], 1.0)
                nc.vector.memset(v_sb[:, :, 129:130], 1.0)
                inv_w = facp.tile([2, 2, N], BF16, tag="inv", name="inv_w")
                t_.update(
                    w=w, raw_w=raw_w, q_rot=q_rot, k_rot=k_rot, v_sb=v_sb,
                    inv_w=inv_w,
                )
                return t_

            def emit_A_t(w, t_, t):
                """Projections + rms factors for tokchunk t of wave w."""
                tsl = slice(t * TOKCH, (t + 1) * TOKCH)
                raw_w, inv_w, v_sb = t_["raw_w"], t_["inv_w"], t_["v_sb"]
                wi = t_["w"]
                for qk in range(2):
                    pj = pp.tile([128, TOKCH], F32, tag="proj", name="pj")
                    for dc in range(NDC):
                        nc.tensor.matmul(
                            pj,
                            lhsT=wqk_sb[:, qk, wi, dc, :],
                            rhs=xt_sb[:, dc, tsl],
                            start=(dc == 0),
                            stop=(dc == NDC - 1),
                        )
                    nc.vector.tensor_copy(raw_w[:, qk, tsl], pj)
                    sq = work.tile([128, TOKCH], BF16, tag="sq")
                    nc.vector.tensor_mul(sq, raw_w[:, qk, tsl], raw_w[:, qk, tsl])
                    ssp = ps.tile([2, TOKCH], F32, tag="s", name="ssp")
                    nc.tensor.matmul(ssp, lhsT=ind2, rhs=sq, start=True, stop=True)
                    lnm = work.tile([2, TOKCH], F32, tag="lnm")
                    nc.scalar.activation(
                        lnm, ssp, ACT.Ln, bias=eps_sb[0:2, :], scale=1.0 / D_HEAD
                    )
                    nc.scalar.activation(
                        inv_w[:, qk, tsl], lnm, ACT.Exp, scale=-0.5
                    )
                # V projection + transpose
                pj = pp.tile([128, TOKCH], F32, tag="proj", name="pjv")
                for dc in range(NDC):
                    nc.tensor.matmul(
                        pj,
                        lhsT=wv_all[:, wi, dc, :],
                        rhs=xt_sb[:, dc, tsl],
                        start=(dc == 0),
                        stop=(dc == NDC - 1),
                    )
                vt = work.tile([128, TOKCH], BF16, tag="vt")
                nc.vector.tensor_copy(vt, pj)
                for sview in range(TOKCH // 128):
                    kb = t * (TOKCH // 128) + sview
                    ptr = pp.tile([128, 128], BF16, tag="proj", name="ptr")
                    nc.tensor.transpose(
                        ptr, vt[:, sview * 128 : (sview + 1) * 128], ident
                    )
                    nc.vector.tensor_copy(v_sb[:, kb, 0:64], ptr[:, 0:64])
                    nc.vector.tensor_copy(v_sb[:, kb, 65:129], ptr[:, 64:128])

            def emit_C_t(w, t_, t):
                """Normalize + rope for tokchunk t."""
                tsl = slice(t * TOKCH, (t + 1) * TOKCH)
                raw_w, inv_w = t_["raw_w"], t_["inv_w"]
                for qk in range(2):
                    rot = t_["q_rot"] if qk == 0 else t_["k_rot"]
                    fac = ps.tile([128, TOKCH], F32, tag="s", name="fac")
                    nc.tensor.matmul(
                        fac, lhsT=wfold, rhs=inv_w[:, qk, tsl],
                        start=True, stop=True,
                    )
                    qn = work.tile([128, TOKCH], BF16, tag="qn")
                    nc.vector.tensor_mul(qn, raw_w[:, qk, tsl], fac)
                    swp = ps.tile([128, TOKCH], F32, tag="s", name="swp")
                    nc.tensor.matmul(
                        swp, lhsT=pswap, rhs=qn, start=True, stop=True
                    )
                    qcos = work.tile([128, TOKCH], BF16, tag="qcos")
                    nc.vector.tensor_mul(qcos, qn, rope_sb[:, 2 * qk, tsl])
                    qsin = work.tile([128, TOKCH], BF16, tag="qsin")
                    nc.vector.tensor_mul(qsin, swp, rope_sb[:, 2 * qk + 1, tsl])
                    nc.vector.tensor_add(rot[:, tsl], qcos, qsin)

            def emit_D_qc(w, t_, qc):
                """Attention for query chunk qc; PV pipelined one kb behind."""
                q_rot, k_rot, v_sb = t_["q_rot"], t_["k_rot"], t_["v_sb"]
                po = [
                    pv.tile([65, TOKCH], F32, tag="pv", name=f"po{h2}")
                    for h2 in range(2)
                ]
                first_kb = [True, True]
                pend = None  # (kb, qlo, es pair)
                kbs = [kb for kb in range(n_kb[qc]) if (qc, kb) in qlo_t]

                def flush_pv(last):
                    kb, qlo, es = pend
                    osl = slice(qlo, TOKCH)
                    for h2 in range(2):
                        nc.tensor.matmul(
                            po[h2][:, osl],
                            lhsT=v_sb[:, kb, 65 * h2 : 65 * h2 + 65],
                            rhs=es[h2][:, osl],
                            start=first_kb[h2],
                            stop=last,
                        )
                        first_kb[h2] = False

                for kb in kbs:
                    qlo = qlo_t[(qc, kb)] * QT
                    csl = slice(qc * TOKCH + qlo, (qc + 1) * TOKCH)
                    osl = slice(qlo, TOKCH)
                    es = [None, None]
                    for h2 in range(2):
                        hr = slice(64 * h2, 64 * h2 + 64)
                        pst = ps.tile([128, TOKCH], F32, tag="s", name="pst")
                        nc.tensor.matmul(
                            pst[:, osl],
                            lhsT=k_rot[hr, kb * KB : (kb + 1) * KB],
                            rhs=q_rot[hr, csl],
                            start=True,
                            stop=True,
                        )
                        e = espool.tile([128, TOKCH], BF16, tag="es", name="es")
                        nc.scalar.activation(
                            e[:, osl], pst[:, osl], ACT.Exp,
                            scale=float(D_HEAD) ** -0.5,
                        )
                        es[h2] = e
                    for j in range(qlo // QT, QPC):
                        st = state[kb][qc * QPC + j]
                        if isinstance(st, int):
                            jsl = slice(j * QT, (j + 1) * QT)
                            for h2 in range(2):
                                nc.vector.tensor_mul(
                                    es[h2][:, jsl], es[h2][:, jsl],
                                    pat_sb[:, st, :],
                                )
                    if pend is not None:
                        flush_pv(False)
                    pend = (kb, qlo, es)
                flush_pv(True)

                # epilogue: rec = exp(-ln(denom)); yt = yraw * bcast(rec)
                for h2 in range(2):
                    lnd = epi.tile([65, TOKCH], F32, tag="lnd", name="lnd")
                    nc.scalar.activation(lnd[64:65, :], po[h2][64:65, :], ACT.Ln)
                    yraw = epi.tile([64, TOKCH], BF16, tag="yraw", name="yraw")
                    nc.vector.tensor_copy(yraw, po[h2][0:64, :])
                    rec = epi.tile([65, TOKCH], BF16, tag="rec", name="rec")
                    nc.scalar.activation(
                        rec[64:65, :], lnd[64:65, :], ACT.Exp, scale=-1.0
                    )
                    f2 = ps.tile([64, TOKCH], F32, tag="s", name="f2")
                    nc.tensor.matmul(
                        f2, lhsT=ones_col[64:65, :], rhs=rec[64:65, :],
                        start=True, stop=True,
                    )
                    nc.vector.tensor_mul(
                        yt_sb[
                            64 * h2 : 64 * h2 + 64, w,
                            qc * TOKCH : (qc + 1) * TOKCH,
                        ],
                        yraw,
                        f2,
                    )

            def emit_out_qc(qc):
                """Out-projection, chunked ReduceScatter and f32 unpack for
                the tokens of query chunk qc (all heads complete there)."""
                for t2 in range(qc * 4, qc * 4 + 4):
                    for ec in range(2):
                        pot = pp.tile([128, TOKCH], F32, tag="proj", name="pot")
                        for fc in range(4):
                            nc.tensor.matmul(
                                pot,
                                lhsT=yt_sb[:, fc, t2 * 128 : (t2 + 1) * 128],
                                rhs=wo_sb[:, fc, ec * TOKCH : (ec + 1) * TOKCH],
                                start=(fc == 0),
                                stop=(fc == 3),
                            )
                        osb = outp.tile([128, TOKCH], BF16, tag="o", name="osb")
                        nc.vector.tensor_copy(osb, pot)
                        r2 = t2 * 128 - qc * TOKCH
                        nc.sync.dma_start(
                            out=y_parts[qc].ap()[
                                r2 : r2 + 128, ec * TOKCH : (ec + 1) * TOKCH
                            ],
                            in_=osb,
                        )
                nc.gpsimd.collective_compute(
                    "ReduceScatter",
                    mybir.AluOpType.add,
                    ins=[y_parts[qc].ap().opt()],
                    outs=[rs_outs[qc].ap().opt()],
                    replica_groups=[[0, 1], [2, 3], [4, 5], [6, 7]],
                )
                for half2 in range(2):
                    rt = outp.tile([128, D_MODEL], BF16, tag="rt", name="rt")
                    nc.gpsimd.dma_start(
                        out=rt,
                        in_=rs_outs[qc].ap()[half2 * 128 : (half2 + 1) * 128, :],
                    )
                    ro = outp.tile([128, D_MODEL], F32, tag="ro", name="ro")
                    nc.vector.tensor_copy(ro, rt)
                    r0 = qc * 256 + half2 * 128
                    nc.gpsimd.dma_start(out=p_out[r0 : r0 + 128, :], in_=ro)

            # software-pipelined emission: proj chunks of wave w+1 are
            # interleaved between attention chunks of wave w to keep the
            # PE stream dense (HAM warm); out-proj + chunked ReduceScatter
            # overlap the last wave.
            for qk in range(2):
                nc.scalar.dma_start(
                    out=wqk_sb[:, qk, 0, :, :], in_=p_wqk[:, qk, 0, :, :]
                )
            nc.scalar.dma_start(out=wv_all[:, 0, :, :], in_=p_wv[:, 0, :, :])
            for w2 in range(1, WAVES):
                for qk in range(2):
                    nc.scalar.dma_start(
                        out=wqk_sb[:, qk, w2, :, :], in_=p_wqk[:, qk, w2, :, :]
                    )
                nc.scalar.dma_start(out=wv_all[:, w2, :, :], in_=p_wv[:, w2, :, :])
            nc.gpsimd.dma_start(out=rope_sb, in_=p_rope)
            if n_patterns:
                nc.gpsimd.dma_start(out=pat_sb, in_=p_pat)
            nc.gpsimd.dma_start(out=wo_sb, in_=p_wo)
            cur = emit_A_head(0)
            for t in range(NT):
                emit_A_t(0, cur, t)
            for w in range(WAVES):
                nxt = emit_A_head(w + 1) if w + 1 < WAVES else None
                for qc in range(NT):
                    emit_C_t(w, cur, qc)
                    if nxt is not None:
                        emit_A_t(w + 1, nxt, qc)
                    emit_D_qc(w, cur, qc)
                    if w == WAVES - 1:
                        emit_out_qc(qc)
                cur = nxt

    nc.compile()
    return nc


def _host_prep(x, mask, pos, W_qkv, W_out, qn_w, kn_w):
    x = np.asarray(x, dtype=np.float32)
    mask = np.asarray(mask)
    pos = np.asarray(pos).astype(np.float64)
    W_qkv = np.asarray(W_qkv, dtype=np.float32)
    W_out = np.asarray(W_out, dtype=np.float32)
    qn_w = np.asarray(qn_w, dtype=np.float32)
    kn_w = np.asarray(kn_w, dtype=np.float32)

    inv_freq = 1.0 / (ROPE_BASE ** (np.arange(0, D_HEAD, 2, dtype=np.float64) / D_HEAD))
    ang = pos[:, None] * inv_freq[None, :]  # (N, 32)
    cosT = np.cos(ang).T.astype(np.float32)  # (32, N)
    sinT = np.sin(ang).T.astype(np.float32)

    def rope_tables(w):
        cos_d = np.tile(cosT, (4, 1)) * np.tile(w, 2)[:, None]
        sin_half = np.concatenate(
            [-sinT * w[32:64][:, None], sinT * w[0:32][:, None]], axis=0
        )
        sin_d = np.tile(sin_half, (2, 1))
        return cos_d, sin_d

    cq, sq = rope_tables(qn_w)
    ck, sk = rope_tables(kn_w)
    rope = np.stack([cq, sq, ck, sk], axis=1).astype(BF)  # (128, 4, N)

    pswap_np = np.zeros((128, 128), dtype=np.float32)
    for a in range(2):
        for r in range(32):
            pswap_np[64 * a + r, 64 * a + 32 + r] = 1.0
            pswap_np[64 * a + 32 + r, 64 * a + r] = 1.0
    pswap_np = pswap_np.astype(BF)

    wfold_np = np.zeros((2, 128), dtype=np.float32)
    wfold_np[0, 0:64] = 1.0
    wfold_np[1, 64:128] = 1.0
    ind2_np = np.ascontiguousarray(wfold_np.T).astype(BF)
    wfold_np = wfold_np.astype(BF)

    state, patterns = _classify_mask(mask)
    if patterns:
        pat = np.stack(patterns, axis=1).astype(BF)
    else:
        pat = None

    q_rows = lambda h: slice(h * 192, h * 192 + 64)
    k_rows = lambda h: slice(h * 192 + 64, h * 192 + 128)
    v_rows = lambda h: slice(h * 192 + 128, h * 192 + 192)

    in_maps = []
    for c in range(N_CORES):
        b, half = divmod(c, 2)
        hs = [8 * half + i for i in range(8)]
        wqk = np.concatenate(
            [W_qkv[q_rows(h)] for h in hs] + [W_qkv[k_rows(h)] for h in hs], axis=0
        ).T  # (1024 dmodel, 1024 cols)
        wv = np.concatenate([W_qkv[v_rows(h)] for h in hs], axis=0).T
        wo = W_out[:, 512 * half : 512 * half + 512].T  # (512, 1024)
        # (128, 2, WAVES, NDC, 128): [p, qk, w, dc, f]
        wqk_re = np.ascontiguousarray(
            wqk.reshape(NDC, 128, 2, WAVES, 128).transpose(1, 2, 3, 0, 4)
        )
        wv_re = np.ascontiguousarray(
            wv.reshape(NDC, 128, WAVES, 128).transpose(1, 2, 0, 3)
        )
        wo_re = np.ascontiguousarray(wo.reshape(4, 128, 1024).transpose(1, 0, 2))
        m = {
            "xt": np.ascontiguousarray(x[b].T).astype(BF),
            "wqk": wqk_re.astype(BF),
            "wv": wv_re.astype(BF),
            "wo": wo_re.astype(BF),
            "rope": rope,
            "wfold": wfold_np,
            "ind2": ind2_np,
            "pswap": pswap_np,
        }
        if pat is not None:
            m["pat"] = pat
        in_maps.append(m)
    return in_maps, state, (0 if pat is None else pat.shape[1])


def kernel(x, mask, pos, W_qkv, W_out, qn_w, kn_w, _trace=False):
    in_maps, state, n_pat = _host_prep(x, mask, pos, W_qkv, W_out, qn_w, kn_w)
    key = (str(state), n_pat)
    if key not in _CACHE:
        _CACHE[key] = _build_program(state, n_pat)
    nc = _CACHE[key]
    res = run_bass_kernel_spmd(nc, in_maps, list(range(N_CORES)), trace=_trace)
    out = np.empty((B, N, D_MODEL), dtype=np.float32)
    for b in range(B):
        lo = res.results[2 * b]["out"]
        hi = res.results[2 * b + 1]["out"]
        for qc in range(NT):
            out[b, qc * TOKCH : qc * TOKCH + 256] = lo[qc * 256 : (qc + 1) * 256]
            out[b, qc * TOKCH + 256 : (qc + 1) * TOKCH] = hi[qc * 256 : (qc + 1) * 256]
    kernel._last_results = res
    return out
